# revision 1
# baseline (speedup 1.0000x reference)
"""Causal self-attention (B=2, T=4096, C=768, H=12) on 8 TRN2 NeuronCores.

Sharding: batch x head-group. Core c handles batch b=c//4 and heads
h0..h0+2 where h0 = 3*(c%4). Each core computes qkv projection for its 3
heads, full causal attention, and a partial output projection; the host
sums the 4 partials per batch and adds the projection bias.

On-chip layout is feature-major ("transposed"): qT/kT [D, T] feed the
scores matmul directly, scores^T [k, q] feeds att@v with v in natural
layout, and the attention output stays transposed to feed the output
projection as the stationary operand (producing natural-layout y).
Matmuls run in float32r (~tf32). The softmax denominator comes free as a
65th "ones" column of v; normalization uses reciprocal_approx_fast + a
gpsimd partition broadcast.
"""

import sys

for _p in ("/opt/trn_rl_repo",):
    if _p not in sys.path:
        sys.path.insert(0, _p)

from contextlib import ExitStack

import numpy as np

import concourse.bass as bass  # noqa: F401  (engine classes referenced via nc)
import concourse.mybir as mybir
import concourse.tile as tile
from concourse import bacc
from concourse.bass_utils import run_bass_kernel_spmd
from concourse.masks import make_identity
from concourse.tile_rust import add_dep_helper

f32 = mybir.dt.float32
f32r = mybir.dt.float32r
AF = mybir.ActivationFunctionType

C = 768
D = 64
N_HEAD = 12
HPC = 3  # heads per core
N_CORES = 8

# wq column slots: q01 | k01 | v01 | (q2 stacked over k2) | v2
SLOTS = [(0, 128), (128, 256), (256, 384), (384, 512), (512, 576)]


def build_nc(T):
    NT = T // 512  # q tiles
    KT = T // 128  # k tiles
    CK = C // 128  # contraction chunks for qkv

    nc = bacc.Bacc("TRN2", target_bir_lowering=False, debug=False,
                   num_devices=N_CORES)
    xt_d = nc.dram_tensor("xt", [C, T], f32r, kind="ExternalInput").ap()
    wq_d = nc.dram_tensor("wq", [C, 576], f32r, kind="ExternalInput").ap()
    bq_d = nc.dram_tensor("bq", [128, 5], f32, kind="ExternalInput").ap()
    wp_d = nc.dram_tensor("wp", [HPC * D, C], f32r, kind="ExternalInput").ap()
    y_d = nc.dram_tensor("y", [T, C], f32, kind="ExternalOutput").ap()
    import os
    dbg = os.environ.get("KDBG") == "1"
    kphase = int(os.environ.get("KPHASE", "4"))
    # internal DRAM scratch for the softmax-reciprocal row broadcast
    rsc_d = nc.dram_tensor("rscratch", [NT * HPC, 512], f32,
                           **({"kind": "ExternalOutput"} if dbg else {})).ap()
    dbg_out = {}
    if dbg:
        for nm, shp in [("d_qAB", [128, T]), ("d_kAB", [128, T]),
                        ("d_qC", [128, T]), ("d_kC", [128, T]),
                        ("d_vaug", [128, KT * 195]),
                        ("d_ao0", [64, T]), ("d_ao1", [64, T]),
                        ("d_ao2", [64, T]), ("d_bc", [64, 512]),
                        ("d_eb", [128, 3072]), ("d_attv", [65, 512])]:
            dbg_out[nm] = nc.dram_tensor(nm, shp, f32, kind="ExternalOutput").ap()

    with tile.TileContext(nc) as tc, ExitStack() as ctx:
        sb = ctx.enter_context(tc.tile_pool(name="sb", bufs=1))

        # persistent tensors (live for the whole kernel)
        bq_sb = sb.tile([128, 5], f32, tag="bq")
        qT_AB = sb.tile([128, T], f32r, tag="qAB")
        kT_AB = sb.tile([128, T], f32r, tag="kAB")
        qT_C = sb.tile([128, T], f32r, tag="qC")
        kT_C = sb.tile([128, T], f32r, tag="kC")
        ident = sb.tile([128, 128], f32, tag="ident")
        ones_f = sb.tile([128, 1], f32, tag="ones")

        nc.sync.dma_start(bq_sb[:], bq_d)
        make_identity(nc, ident[:])
        nc.vector.memset(ones_f[:], 1.0)
        # causal masks for the 4 diagonal-band positions: keep col-p >= 128*r
        cmask = sb.tile([128, 4 * 512], f32, tag="cmask")
        nc.gpsimd.memset(cmask[:], 1.0)
        for r in range(4):
            nc.gpsimd.affine_select(
                cmask[:, r * 512:(r + 1) * 512], cmask[:, r * 512:(r + 1) * 512],
                pattern=[[1, 512]], compare_op=mybir.AluOpType.is_ge, fill=0.0,
                base=-128 * r, channel_multiplier=-1)

        # vaug lives phases 2-3; vpool (inside it) only phases 1-2
        vaugp = ctx.enter_context(tc.tile_pool(name="vaugp", bufs=1))
        es_v = ExitStack()
        vp = es_v.enter_context(tc.tile_pool(name="vpool", bufs=1))
        vT01 = vp.tile([128, T], f32, tag="v01")
        vT2 = vp.tile([64, T], f32, tag="v2")

        # ---------------- phase 1: qkv projection (transposed) --------------
        with tc.tile_pool(name="wqp", bufs=1) as wqp, \
             tc.tile_pool(name="xtp", bufs=12) as xt_pool, \
             tc.tile_pool(name="qkvpsA", bufs=2, space="PSUM") as qkv_psA, \
             tc.tile_pool(name="qkvps", bufs=1, space="PSUM") as qkv_ps:
            wq_sb = [wqp.tile([128, 576], f32r, tag=f"wq{c}", name=f"wq{c}")
                     for c in range(CK)]
            for c in range(CK):
                nc.sync.dma_start(wq_sb[c][:], wq_d[c * 128:(c + 1) * 128, :])
            for j in range(NT):
                jsl = bass.ts(j, 512)
                ps = [qkv_psA.tile([128, 512], f32, tag=f"s{k}", name=f"ps{k}")
                      for k in range(3)]
                ps.append(qkv_ps.tile([128, 512], f32, tag="s3", name="ps3"))
                ps.append(qkv_ps.tile([64, 512], f32, tag="s4", name="ps4"))
                for c in range(CK):
                    xt_t = xt_pool.tile([128, 512], f32r, tag="xt")
                    nc.sync.dma_start(
                        xt_t[:], xt_d[c * 128:(c + 1) * 128, j * 512:(j + 1) * 512])
                    for s, (c0, c1) in enumerate(SLOTS):
                        nc.tensor.matmul(ps[s][:], wq_sb[c][:, c0:c1], xt_t[:],
                                         start=(c == 0), stop=(c == CK - 1))
                nc.vector.tensor_scalar_add(qT_AB[:, jsl], ps[0][:], bq_sb[:, 0:1])
                nc.vector.tensor_scalar_add(kT_AB[:, jsl], ps[1][:], bq_sb[:, 1:2])
                nc.vector.tensor_scalar_add(vT01[:, jsl], ps[2][:], bq_sb[:, 2:3])
                nc.vector.tensor_scalar_add(qT_C[0:64, jsl], ps[3][0:64, :],
                                            bq_sb[0:64, 3:4])
                nc.vector.tensor_scalar_add(kT_C[64:128, jsl], ps[3][64:128, :],
                                            bq_sb[64:128, 3:4])
                nc.vector.tensor_scalar_add(vT2[:, jsl], ps[4][:], bq_sb[0:64, 4:5])
            # duplicate head-2 q/k into the other 64-partition strip
            nc.sync.dma_start(qT_C[64:128, :], qT_C[0:64, :])
            nc.sync.dma_start(kT_C[0:64, :], kT_C[64:128, :])
            if dbg:
                nc.sync.dma_start(dbg_out["d_qAB"], qT_AB[:].bitcast(f32))
                nc.sync.dma_start(dbg_out["d_kAB"], kT_AB[:].bitcast(f32))
                nc.sync.dma_start(dbg_out["d_qC"], qT_C[:].bitcast(f32))
                nc.sync.dma_start(dbg_out["d_kC"], kT_C[:].bitcast(f32))

        # ---------------- phase 2: v -> natural layout + ones column --------
        if kphase >= 2:
          v_aug = vaugp.tile([128, KT * 195], f32r, tag="vaug")
          with tc.tile_pool(name="tps", bufs=3, space="PSUM") as tp_ps:
            for ki in range(KT):
                ksl = bass.ts(ki, 128)
                base = ki * 195
                p01 = tp_ps.tile([128, 128], f32, tag="tp01")
                nc.tensor.transpose(p01[:], vT01[:, ksl], ident[:])
                p2t = tp_ps.tile([128, 64], f32, tag="tp2")
                nc.tensor.transpose(p2t[:], vT2[:, ksl], ident[0:64, 0:64])
                nc.vector.tensor_copy(v_aug[:, base:base + 64], p01[:, 0:64])
                nc.vector.tensor_copy(v_aug[:, base + 65:base + 129], p01[:, 64:128])
                nc.vector.tensor_copy(v_aug[:, base + 130:base + 194], p2t[:])
            ones_cols = v_aug[:].rearrange("p (k c) -> p k c", c=65)[:, :, 64:65]
            nc.vector.tensor_copy(
                ones_cols, ones_f[:, 0:1, None].broadcast_to([128, 3 * KT, 1]))
          if dbg:
              nc.sync.dma_start(dbg_out["d_vaug"], v_aug[:].bitcast(f32))
          es_v.close()  # vT buffers no longer needed

          # ---------------- phase 3: attention -------------------------------
          aop = ctx.enter_context(tc.tile_pool(name="aop", bufs=1))
          aoT = [aop.tile([64, T], f32r, tag=f"aoT{h}", name=f"aoT{h}")
                 for h in range(HPC)]
          with tc.tile_pool(name="scps", bufs=2, space="PSUM") as sc_ps, \
             tc.tile_pool(name="avps", bufs=3, space="PSUM") as av_ps, \
             tc.tile_pool(name="pps", bufs=1, space="PSUM") as pr_ps, \
             tc.tile_pool(name="ebp", bufs=6) as eb_pool, \
             tc.tile_pool(name="wpp", bufs=1) as wpp, \
             tc.tile_pool(name="yp", bufs=3) as y_pool, \
             tc.tile_pool(name="nrm", bufs=3) as nrm:
            wp_sb = [wpp.tile([64, C], f32r, tag=f"wp{h}", name=f"wp{h}")
                     for h in range(HPC)]
            for h in range(HPC):
                nc.sync.dma_start(wp_sb[h][:], wp_d[h * 64:(h + 1) * 64, :])

            def emit_proj(m):
                msl = bass.ts(m, 128)
                y_sb = y_pool.tile([128, C], f32, tag="y", name="ysb")
                for ns in range(2):
                    py = pr_ps.tile([128, 384], f32, tag="py", name="py")
                    for h in range(HPC):
                        nc.tensor.matmul(py[:], aoT[h][:, msl],
                                         wp_sb[h][:, ns * 384:(ns + 1) * 384],
                                         start=(h == 0), stop=(h == HPC - 1))
                    nc.vector.tensor_copy(y_sb[:, ns * 384:(ns + 1) * 384],
                                          py[:])
                nc.sync.dma_start(y_d[m * 128:(m + 1) * 128, :], y_sb[:])

            for j in range(NT if kphase >= 3 else 0):
                jsl = bass.ts(j, 512)
                nk = 4 * j + 4
                for slot in ("AB", "C"):
                    if slot == "AB":
                        heads = [0, 1]
                        group = 1  # k-tiles per round (2 banks each)
                    else:
                        heads = [2]
                        group = 2
                    att = {h: av_ps.tile([65, 512], f32, tag="attv", name=f"attv{h}")
                           for h in heads}
                    for g0 in range(0, nk, group):
                        ks = list(range(g0, min(g0 + group, nk)))
                        nbank = len(ks) * len(heads)
                        pr = sc_ps.tile([128, 1024], f32, tag="sc")
                        banks = []  # (bank, ki, head)
                        for idx, ki in enumerate(ks):
                            ksl = bass.ts(ki, 128)
                            if slot == "AB":
                                for hh in (0, 1):
                                    b = idx * 2 + hh
                                    r0, r1 = 64 * hh, 64 * hh + 64
                                    nc.tensor.matmul(
                                        pr[:, bass.ts(b, 512)],
                                        kT_AB[r0:r1, ksl], qT_AB[r0:r1, jsl],
                                        start=True, stop=True)
                                    banks.append((b, ki, hh))
                            else:
                                strip = idx % 2
                                r0, r1 = 64 * strip, 64 * strip + 64
                                nc.tensor.matmul(
                                    pr[:, bass.ts(idx, 512)],
                                    kT_C[r0:r1, ksl], qT_C[r0:r1, jsl],
                                    start=True, stop=True)
                                banks.append((idx, ki, 2))
                        eb = eb_pool.tile([128, 1024], f32r, tag="eb")
                        nc.scalar.activation(eb[:, 0:nbank * 512],
                                             pr[:, 0:nbank * 512],
                                             AF.Exp, scale=0.125)
                        if dbg and j == NT - 1 and slot == "C" and g0 == 0:
                            nc.sync.dma_start(dbg_out["d_eb"][:, 0:nbank * 512],
                                              eb[:, 0:nbank * 512].bitcast(f32))
                        for b, ki, h in banks:
                            if ki >= 4 * j:  # diagonal band: causal mask
                                bsl = bass.ts(b, 512)
                                r = ki - 4 * j
                                nc.vector.tensor_mul(
                                    eb[:, bsl], eb[:, bsl],
                                    cmask[:, bass.ts(r, 512)])
                        for b, ki, h in banks:
                            nc.tensor.matmul(
                                att[h][:], v_aug[:, ki * 195 + 65 * h:
                                                 ki * 195 + 65 * h + 65],
                                eb[:, bass.ts(b, 512)],
                                start=(ki == 0), stop=(ki == nk - 1),
                                skip_group_check=True)
                    for h in heads:
                        if dbg and j == NT - 1 and h == 2:
                            datt = nrm.tile([65, 512], f32, tag="datt")
                            nc.vector.tensor_copy(datt[:], att[h][:])
                            nc.sync.dma_start(dbg_out["d_attv"], datt[:])
                        # denominator row (psum p64) -> sbuf, then broadcast
                        # across 64 partitions via a DRAM round-trip (stride-0
                        # leading dim is DRAM-only). Tile does not dep-track
                        # DRAM, so wire the RAW edge explicitly. The recip runs
                        # after the broadcast: custom-dve ops misbehave at
                        # nonzero base partitions.
                        scrA = nrm.tile([65, 512], f32, tag="scrA")
                        nc.vector.tensor_copy(scrA[64:65, :], att[h][64:65, :])
                        row_d = rsc_d[j * HPC + h, :]
                        wr = nc.sync.dma_start(row_d[None, :], scrA[64:65, :])
                        bc = nrm.tile([64, 512], f32, tag="bc")
                        rd = nc.gpsimd.dma_start(
                            out=bc[:], in_=bass.AP(row_d.tensor, row_d.offset,
                                                   [[0, 64], [1, 512]]))
                        add_dep_helper(rd.ins, wr.ins,
                                       reason="rscratch RAW (dram roundtrip)")
                        rcp = nrm.tile([64, 512], f32, tag="rcp")
                        nc.vector.reciprocal_approx_fast(out=rcp[:], in_=bc[:])
                        nc.vector.tensor_mul(aoT[h][:, jsl], att[h][0:64, :], rcp[:])
                        if dbg and j == NT - 1 and h == 2:
                            nc.sync.dma_start(dbg_out["d_bc"], bc[:])
                for m in range(4 * j, 4 * j + 4):
                    if kphase >= 4:
                        emit_proj(m)

        if dbg:
            for h in range(HPC):
                nc.sync.dma_start(dbg_out[f"d_ao{h}"], aoT[h][:].bitcast(f32))

    nc.compile()
    return nc


_NC_CACHE = {}


def _get_nc(T):
    if T not in _NC_CACHE:
        _NC_CACHE[T] = build_nc(T)
    return _NC_CACHE[T]


def make_core_inputs(x, W_attn, b_attn, W_proj):
    """Host-side prep: per-core input dicts (see module docstring)."""
    B, T, _ = x.shape
    xts = [np.ascontiguousarray(x[b].T) for b in range(B)]
    in_maps = []
    for core in range(N_CORES):
        b = core // (N_CORES // B)
        h0 = HPC * (core % (N_CORES // B))
        ccols = slice(h0 * D, (h0 + 2) * D)      # first two heads
        c2 = slice((h0 + 2) * D, (h0 + 3) * D)   # third head
        # reference splits qkv as (k, q, v): k cols 0:C, q cols C:2C, v 2C:3C
        q01 = W_attn[:, C:2 * C][:, ccols]
        k01 = W_attn[:, 0:C][:, ccols]
        v01 = W_attn[:, 2 * C:3 * C][:, ccols]
        q2 = W_attn[:, C:2 * C][:, c2]
        k2 = W_attn[:, 0:C][:, c2]
        v2 = W_attn[:, 2 * C:3 * C][:, c2]
        wq = np.ascontiguousarray(
            np.concatenate([q01, k01, v01, q2, k2, v2], axis=1))
        bq = np.zeros((128, 5), np.float32)
        bq[:, 0] = b_attn[C:2 * C][ccols]
        bq[:, 1] = b_attn[0:C][ccols]
        bq[:, 2] = b_attn[2 * C:3 * C][ccols]
        bq[0:64, 3] = b_attn[C:2 * C][c2]
        bq[64:128, 3] = b_attn[0:C][c2]
        bq[0:64, 4] = b_attn[2 * C:3 * C][c2]
        wp = np.ascontiguousarray(W_proj[h0 * D:(h0 + HPC) * D, :])
        in_maps.append({"xt": xts[b], "wq": wq, "bq": bq, "wp": wp})
    return in_maps


def kernel(x, W_attn, b_attn, W_proj, b_proj):
    x = np.asarray(x, dtype=np.float32)
    W_attn = np.asarray(W_attn, dtype=np.float32)
    b_attn = np.asarray(b_attn, dtype=np.float32)
    W_proj = np.asarray(W_proj, dtype=np.float32)
    b_proj = np.asarray(b_proj, dtype=np.float32)
    B, T, _ = x.shape

    nc = _get_nc(T)
    in_maps = make_core_inputs(x, W_attn, b_attn, W_proj)
    res = None
    for attempt in range(3):
        try:
            res = run_bass_kernel_spmd(nc, in_maps, list(range(N_CORES)))
            break
        except Exception:
            # transient NRT_EXEC_UNIT_UNRECOVERABLE has been observed once
            # after a prior crashed process; a retry succeeds
            if attempt == 2:
                raise
    global LAST_RUN
    LAST_RUN = res

    gpb = N_CORES // B
    out = np.empty((B, T, C), np.float32)
    for b in range(B):
        acc = res.results[b * gpb]["y"].astype(np.float32)
        for g in range(1, gpb):
            acc = acc + res.results[b * gpb + g]["y"]
        out[b] = acc + b_proj[None, :]
    return out



# revision 9
# speedup vs baseline: 1.1428x; 1.1428x over previous
"""Causal self-attention (B=2, T=4096, C=768, H=12) on 8 TRN2 NeuronCores.

Sharding: batch x head-group. Core c handles batch b=c//4 and heads
h0..h0+2 where h0 = 3*(c%4). Each core computes the qkv projection for
its 3 heads, full causal attention, and a partial output projection; the
host sums the 4 partials per batch and adds the (augmented) projection
bias.

Numerics / structure:
- All matmul operands are bf16 (f32 PSUM accumulation).
- k-bias is dropped entirely: softmax over k is invariant to the
  per-query constant q . b_k. v-bias is folded into the host-side output
  bias (softmax weights sum to 1, so + b_v @ W_proj).
- q/k live transposed ([d, T]) feeding the scores matmul; v is computed
  directly in natural layout ([T, d]) with an interleaved ones column
  per head providing the softmax denominator through the att@v matmul.
- scores^T tiles [128 k, 512 q] -> exp -> eb (bf16). exp runs on the Act
  engine for ~55% of tiles and as a Schraudolph int16-bitcast
  approximation (tensor_scalar mult+add, then f32->int16 convert copy)
  on gpsimd+DVE for the rest, keeping all three engines busy.
- att@v is "flipped": out [128 q, 65] accumulated over k-tiles (65
  cycles per matmul instead of 512), using eb chunks as the stationary
  operand. Diagonal tiles skip fully-masked chunks/columns.
- Normalization uses the denominator column as a per-partition scalar
  (reciprocal + tensor_scalar_mul), then PE transposes the normalized
  [q, d] chunks to d-major for the output projection.
"""

import sys

for _p in ("/opt/trn_rl_repo",):
    if _p not in sys.path:
        sys.path.insert(0, _p)

import math
from contextlib import ExitStack

import numpy as np
import ml_dtypes

import concourse.bass as bass
import concourse.mybir as mybir
import concourse.tile as tile
from concourse import bacc
from concourse.bass_utils import run_bass_kernel_spmd
from concourse.masks import make_identity

f32 = mybir.dt.float32
bf16 = mybir.dt.bfloat16
i16 = mybir.dt.int16
AF = mybir.ActivationFunctionType
Alu = mybir.AluOpType

C = 768
D = 64
HPC = 3  # heads per core
N_CORES = 8
CK = C // 128  # contraction chunks

SCALE = 1.0 / math.sqrt(D)  # 0.125
# Schraudolph: exp(s*SCALE) ~ bitcast_bf16(int16(s*A_SCH + B_SCH))
A_SCH = SCALE * 128.0 / math.log(2.0)
B_SCH = 128.0 * (127.0 - 0.043)

# exp path pattern over score tiles: True -> Act exp, False -> DVE
# Schraudolph (2 DVE ops). ~7/9 on Act.
EXP_PAT = [True, True, True, True, False, True, True, True, False]


def build_nc(T):
    NT = T // 512  # q tiles
    KT = T // 128  # k tiles
    T2 = T // 2

    nc = bacc.Bacc("TRN2", target_bir_lowering=False, debug=False,
                   num_devices=N_CORES)
    xt_d = nc.dram_tensor("xt", [C, T], bf16, kind="ExternalInput").ap()
    wq_d = nc.dram_tensor("wq", [C, 576], bf16, kind="ExternalInput").ap()
    bq_d = nc.dram_tensor("bq", [128, 2], f32, kind="ExternalInput").ap()
    wp01_d = nc.dram_tensor("wp01", [128, C], bf16, kind="ExternalInput").ap()
    wp2_d = nc.dram_tensor("wp2", [64, C], bf16, kind="ExternalInput").ap()
    y_d = nc.dram_tensor("y", [T, C], f32, kind="ExternalOutput").ap()
    import os
    dbg = os.environ.get("KDBG") == "1"
    dbg_out = {}
    if dbg:
        KT_ = T // 128
        for nm, shp in [("d_qAB", [128, T]), ("d_kAB", [128, T]),
                        ("d_qC", [64, T]), ("d_kC2", [64, T]),
                        ("d_vaug", [128, KT_ * 195]),
                        ("d_ao01", [128, T]), ("d_ao2", [64, T]),
                        ("d_eb", [128, 1024]), ("d_att", [128, 512])]:
            dbg_out[nm] = nc.dram_tensor(nm, shp, f32, kind="ExternalOutput").ap()

    with tile.TileContext(nc) as tc, ExitStack() as ctx:
        sb = ctx.enter_context(tc.tile_pool(name="sb", bufs=1))

        # persistent tensors
        bq_sb = sb.tile([128, 2], f32, tag="bq")
        qT_AB = sb.tile([128, T], bf16, tag="qAB")
        kT_AB = sb.tile([128, T], bf16, tag="kAB")
        qT_C = sb.tile([64, T], bf16, tag="qC")
        kC2 = sb.tile([64, T], bf16, tag="kC2")
        v_aug = sb.tile([128, KT * 195], bf16, tag="vaug")
        aoT01 = sb.tile([128, T], bf16, tag="aoT01")
        aoT2 = sb.tile([64, T], bf16, tag="aoT2")
        ident = sb.tile([128, 128], bf16, tag="ident")
        cmask = sb.tile([128, 128], bf16, tag="cmask")

        nc.sync.dma_start(bq_sb[:], bq_d)
        make_identity(nc, ident[:])
        # causal triangle for the in-diagonal 128-col strip: keep col >= part
        nc.gpsimd.memset(cmask[:], 1.0)
        nc.gpsimd.affine_select(
            cmask[:], cmask[:], pattern=[[1, 128]],
            compare_op=Alu.is_ge, fill=0.0, base=0, channel_multiplier=-1)
        # ones columns of v_aug (denominator rows through att@v)
        ones_cols = v_aug[:].rearrange("p (k h c) -> p k h c", h=3, c=65)[:, :, :, 64:65]
        nc.gpsimd.memset(ones_cols, 1.0)

        wpp = ctx.enter_context(tc.tile_pool(name="wpp", bufs=1))
        wp01 = wpp.tile([128, C], bf16, tag="wp01")
        wp2 = wpp.tile([64, C], bf16, tag="wp2")

        es_p1 = ExitStack()
        xtp = es_p1.enter_context(tc.tile_pool(name="xtp", bufs=1))
        wqp = es_p1.enter_context(tc.tile_pool(name="wqp", bufs=1))
        k2p = es_p1.enter_context(tc.tile_pool(name="k2p", bufs=1))
        xt_sb = [xtp.tile([128, T], bf16, tag=f"xt{c}", name=f"xt{c}")
                 for c in range(CK)]
        wq_sb = [wqp.tile([128, 576], bf16, tag=f"wq{c}", name=f"wq{c}")
                 for c in range(CK)]
        k2s = k2p.tile([128, T], bf16, tag="k2s")
        for c in range(CK):
            nc.sync.dma_start(wq_sb[c][:], wq_d[c * 128:(c + 1) * 128, :])
        for half in range(2):
            hsl = bass.ts(half, T2)
            for c in range(CK):
                nc.sync.dma_start(xt_sb[c][:, hsl],
                                  xt_d[c * 128:(c + 1) * 128, hsl])
        nc.sync.dma_start(wp01[:], wp01_d)
        nc.sync.dma_start(wp2[:], wp2_d)

        # ---------------- phase 1: qkv projection --------------------------
        with tc.tile_pool(name="qkp", bufs=2, space="PSUM") as qkp, \
             tc.tile_pool(name="vps", bufs=2, space="PSUM") as vps:
            for j in range(NT):
                jsl = bass.ts(j, 512)
                q01 = qkp.tile([128, 512], f32, tag="q01", name="q01")
                k01 = qkp.tile([128, 512], f32, tag="k01", name="k01")
                qk2 = qkp.tile([128, 512], f32, tag="qk2", name="qk2")
                for c in range(CK):
                    st, sp = c == 0, c == CK - 1
                    nc.tensor.matmul(q01[:], wq_sb[c][:, 0:128],
                                     xt_sb[c][:, jsl], start=st, stop=sp)
                    nc.tensor.matmul(k01[:], wq_sb[c][:, 128:256],
                                     xt_sb[c][:, jsl], start=st, stop=sp)
                    nc.tensor.matmul(qk2[:], wq_sb[c][:, 256:384],
                                     xt_sb[c][:, jsl], start=st, stop=sp)
                nc.vector.tensor_scalar_add(qT_AB[:, jsl], q01[:],
                                            bq_sb[:, 0:1])
                nc.vector.tensor_copy(kT_AB[:, jsl], k01[:])
                nc.vector.tensor_scalar_add(qT_C[0:64, jsl], qk2[0:64, :],
                                            bq_sb[0:64, 1:2])
                nc.vector.tensor_copy(k2s[64:128, jsl], qk2[64:128, :])
                for mi in range(4):
                    m = 4 * j + mi
                    msl = bass.ts(m, 128)
                    vp = vps.tile([128, 192], f32, tag="vp", name="vp")
                    for c in range(CK):
                        nc.tensor.matmul(vp[:], xt_sb[c][:, msl],
                                         wq_sb[c][:, 384:576],
                                         start=(c == 0), stop=(c == CK - 1))
                    vdst = v_aug[:, m * 195:(m + 1) * 195].rearrange(
                        "p (h c) -> p h c", c=65)[:, :, 0:64]
                    nc.vector.tensor_copy(
                        vdst, vp[:].rearrange("p (h c) -> p h c", c=64))
        # k2: partition shift 64:128 -> 0:64 via sbuf->sbuf DMA
        nc.sync.dma_start(kC2[0:64, :], k2s[64:128, :])
        es_p1.close()
        if dbg:
            dsc = sb
            for nm, t_ in [("d_qAB", qT_AB), ("d_kAB", kT_AB),
                           ("d_vaug", v_aug)]:
                tmp = dsc.tile(list(t_.shape), f32, tag=f"t{nm}", name=f"t{nm}")
                nc.vector.tensor_copy(tmp[:], t_[:])
                nc.sync.dma_start(dbg_out[nm], tmp[:])
            for nm, t_ in [("d_qC", qT_C), ("d_kC2", kC2)]:
                tmp = dsc.tile([64, T], f32, tag=f"t{nm}", name=f"t{nm}")
                nc.vector.tensor_copy(tmp[0:64, :], t_[0:64, :])
                nc.sync.dma_start(dbg_out[nm], tmp[0:64, :])

        # ---------------- phase 3: attention + projection ------------------
        with tc.tile_pool(name="scp", bufs=2, space="PSUM") as scp, \
             tc.tile_pool(name="attp", bufs=2, space="PSUM") as attp, \
             tc.tile_pool(name="trp", bufs=1, space="PSUM") as trp, \
             tc.tile_pool(name="ebp", bufs=4) as ebp, \
             tc.tile_pool(name="ebfp", bufs=2) as ebfp, \
             tc.tile_pool(name="anp", bufs=2) as anp, \
             tc.tile_pool(name="rcpp", bufs=4) as rcpp, \
             tc.tile_pool(name="ysp", bufs=3) as ysp:

            # 2 banks of manually sub-allocated scratch: transposes (bf16
            # views) + projection psum (pyA/pyB rotate). Tile dep-tracking is
            # range-precise, so disjoint sub-ranges don't serialize.
            trx = trp.tile([128, 1024], f32, tag="trx", name="trx")
            trx_bf = trx[:, 0:512].bitcast(bf16)  # [128, 1024] bf16
            exp_ctr = [0]

            def emit_exp(sc_t, eb_t, specs):
                """specs: list of (sc_ap, eb_ap, ebf_cols) with matching
                shapes; one program step, routed to Act or gpsimd+DVE."""
                use_act = EXP_PAT[exp_ctr[0] % len(EXP_PAT)]
                exp_ctr[0] += 1
                if use_act:
                    for sc_ap, eb_ap, _ in specs:
                        nc.scalar.activation(eb_ap, sc_ap, AF.Exp,
                                             scale=SCALE)
                else:
                    ebf = ebfp.tile([128, 1024], f32, tag="ebf", name="ebf")
                    for sc_ap, eb_ap, cols in specs:
                        ebf_ap = ebf[:, cols]
                        if len(sc_ap.shape) == 3:
                            ebf_ap = ebf_ap.rearrange(
                                "p (h c) -> p h c", h=sc_ap.shape[1])
                        nc.vector.tensor_scalar(ebf_ap, sc_ap,
                                                A_SCH, B_SCH,
                                                Alu.mult, Alu.add)
                        nc.vector.tensor_copy(eb_ap.bitcast(i16), ebf_ap)

            proj_pending = []
            py_ctr = [0]

            def emit_proj():
                if not proj_pending:
                    return
                m = proj_pending.pop(0)
                msl = bass.ts(m, 128)
                y_sb = ysp.tile([128, C], f32, tag="ysb", name="ysb")
                for third in range(3):
                    off = 512 + 256 * (py_ctr[0] % 2)
                    py_ctr[0] += 1
                    py = trx[:, off:off + 256]
                    csl = slice(third * 256, (third + 1) * 256)
                    nc.tensor.matmul(py, aoT01[:, msl], wp01[:, csl],
                                     start=True, stop=False,
                                     skip_group_check=True)
                    nc.tensor.matmul(py, aoT2[0:64, msl], wp2[0:64, csl],
                                     start=False, stop=True,
                                     skip_group_check=True)
                    nc.vector.tensor_copy(y_sb[:, csl], py)
                nc.sync.dma_start(y_d[m * 128:(m + 1) * 128, :], y_sb[:])

            for j in range(NT):
                nk = 4 * j + 4
                jsl = bass.ts(j, 512)

                # ---- heads 0,1
                att01 = [attp.tile([128, 512], f32, tag="att",
                                   name=f"att{h}") for h in range(2)]
                for h in range(2):
                    nc.vector.memset(att01[h][:, 0:260], 0.0)
                for ki in range(nk):
                    r = ki - 4 * j  # >= 0 -> diagonal band
                    ksl = bass.ts(ki, 128)
                    trim = 128 * r if r >= 0 else 0
                    w = 512 - trim
                    sc = scp.tile([128, 1024], f32, tag="sc", name="sc")
                    for h in range(2):
                        hp = slice(64 * h, 64 * h + 64)
                        nc.tensor.matmul(
                            sc[:, 512 * h + trim:512 * h + 512],
                            kT_AB[hp, ksl],
                            qT_AB[hp, j * 512 + trim:(j + 1) * 512],
                            start=True, stop=True)
                    eb = ebp.tile([128, 1024], bf16, tag="eb", name="eb")
                    sc_ap = sc[:].rearrange("p (h c) -> p h c", h=2)[:, :, trim:512]
                    eb_ap = eb[:].rearrange("p (h c) -> p h c", h=2)[:, :, trim:512]
                    spec_cols = slice(0, 2 * w)
                    emit_exp(sc, eb, [(sc_ap, eb_ap, spec_cols)])
                    if r >= 0 and r < 4:
                        for h in range(2):
                            strip = slice(512 * h + trim, 512 * h + trim + 128)
                            nc.gpsimd.tensor_mul(eb[:, strip], eb[:, strip],
                                                 cmask[:])
                    for h in range(2):
                        for c4 in range(4):
                            if r >= 0 and c4 < r:
                                continue
                            nc.tensor.matmul(
                                att01[h][:, c4 * 65:c4 * 65 + 65],
                                eb[:, 512 * h + 128 * c4:512 * h + 128 * c4 + 128],
                                v_aug[:, ki * 195 + 65 * h:ki * 195 + 65 * h + 65],
                                start=False, stop=(ki == 4 * j + c4),
                                skip_group_check=True)
                    if dbg and j == 0 and ki == 0:
                        ebt = ebp.tile([128, 1024], f32, tag="ebt", name="ebt")
                        nc.vector.tensor_copy(ebt[:], eb[:])
                        nc.sync.dma_start(dbg_out["d_eb"], ebt[:])
                    if ki % 2 == 1:
                        emit_proj()

                if dbg and j == 0:
                    att_t = ebp.tile([128, 512], f32, tag="attt", name="attt")
                    nc.vector.tensor_copy(att_t[:], att01[0][:])
                    nc.sync.dma_start(dbg_out["d_att"], att_t[:])
                # normalize + transpose heads 0,1
                an01 = anp.tile([128, 512], bf16, tag="an01", name="an01")
                for h in range(2):
                    rcp = rcpp.tile([128, 4], f32, tag="rcp", name="rcp")
                    at = att01[h][:]
                    den = bass.AP(at.tensor, at.offset + 64, [at.ap[0], [65, 4]])
                    nc.vector.reciprocal_approx_fast(out=rcp[:], in_=den)
                    for c4 in range(4):
                        nc.vector.tensor_scalar_mul(
                            an01[:, c4 * 128 + 64 * h:c4 * 128 + 64 * h + 64],
                            att01[h][:, c4 * 65:c4 * 65 + 64],
                            rcp[:, c4:c4 + 1])
                tr01 = trx_bf[:, 0:512]
                for c4 in range(4):
                    csl = bass.ts(c4, 128)
                    nc.tensor.transpose(tr01[:, csl], an01[:, csl], ident[:])
                nc.vector.tensor_copy(aoT01[:, jsl], tr01)

                # ---- head 2 (pairs of k-tiles per psum tile)
                att2 = attp.tile([128, 512], f32, tag="att", name="att2")
                nc.vector.memset(att2[:, 0:260], 0.0)
                for kp in range(nk // 2):
                    sc = scp.tile([128, 1024], f32, tag="sc", name="sc2")
                    eb = ebp.tile([128, 1024], bf16, tag="eb", name="eb2")
                    kis = (2 * kp, 2 * kp + 1)
                    specs = []
                    for half, ki in enumerate(kis):
                        r = ki - 4 * j
                        ksl = bass.ts(ki, 128)
                        trim = 128 * r if r >= 0 else 0
                        nc.tensor.matmul(
                            sc[:, 512 * half + trim:512 * half + 512],
                            kC2[0:64, ksl],
                            qT_C[0:64, j * 512 + trim:(j + 1) * 512],
                            start=True, stop=True)
                        specs.append(
                            (sc[:, 512 * half + trim:512 * half + 512],
                             eb[:, 512 * half + trim:512 * half + 512],
                             slice(512 * half + trim, 512 * half + 512)))
                    if specs[0][2] == slice(0, 512) and \
                       specs[1][2] == slice(512, 1024):
                        specs = [(sc[:], eb[:], slice(0, 1024))]
                    emit_exp(sc, eb, specs)
                    for half, ki in enumerate(kis):
                        r = ki - 4 * j
                        if 0 <= r < 4:
                            trim = 128 * r
                            strip = slice(512 * half + trim,
                                          512 * half + trim + 128)
                            nc.gpsimd.tensor_mul(eb[:, strip], eb[:, strip],
                                                 cmask[:])
                    for half, ki in enumerate(kis):
                        r = ki - 4 * j
                        for c4 in range(4):
                            if r >= 0 and c4 < r:
                                continue
                            nc.tensor.matmul(
                                att2[:, c4 * 65:c4 * 65 + 65],
                                eb[:, 512 * half + 128 * c4:
                                   512 * half + 128 * c4 + 128],
                                v_aug[:, ki * 195 + 130:ki * 195 + 195],
                                start=False, stop=(ki == 4 * j + c4),
                                skip_group_check=True)

                # normalize + transpose head 2
                an2 = anp.tile([128, 256], bf16, tag="an2", name="an2")
                rcp2 = rcpp.tile([128, 4], f32, tag="rcp", name="rcp2")
                at2 = att2[:]
                den2 = bass.AP(at2.tensor, at2.offset + 64, [at2.ap[0], [65, 4]])
                nc.vector.reciprocal_approx_fast(out=rcp2[:], in_=den2)
                for c4 in range(4):
                    nc.vector.tensor_scalar_mul(
                        an2[:, c4 * 64:c4 * 64 + 64],
                        att2[:, c4 * 65:c4 * 65 + 64],
                        rcp2[:, c4:c4 + 1])
                tr2 = trx_bf[0:64, 512:1024]
                for c4 in range(4):
                    nc.tensor.transpose(tr2[:, bass.ts(c4, 128)],
                                        an2[:, bass.ts(c4, 64)],
                                        ident[:])
                nc.vector.tensor_copy(aoT2[0:64, jsl], tr2)

                proj_pending.extend(range(4 * j, 4 * j + 4))

            while proj_pending:
                emit_proj()
            if dbg:
                dao = sb
                t1 = dao.tile([128, T], f32, tag="tao01")
                nc.vector.tensor_copy(t1[:], aoT01[:])
                nc.sync.dma_start(dbg_out["d_ao01"], t1[:])
                t2 = dao.tile([64, T], f32, tag="tao2")
                nc.vector.tensor_copy(t2[0:64, :], aoT2[0:64, :])
                nc.sync.dma_start(dbg_out["d_ao2"], t2[0:64, :])

    nc.compile()
    return nc


_NC_CACHE = {}


def _get_nc(T):
    if T not in _NC_CACHE:
        _NC_CACHE[T] = build_nc(T)
    return _NC_CACHE[T]


def make_core_inputs(x, W_attn, b_attn, W_proj):
    """Host-side prep: per-core input dicts (see module docstring)."""
    B, T, _ = x.shape
    xts = [np.ascontiguousarray(x[b].T).astype(ml_dtypes.bfloat16)
           for b in range(B)]
    # reference splits qkv as (k, q, v)
    Wk, Wq, Wv = W_attn[:, 0:C], W_attn[:, C:2 * C], W_attn[:, 2 * C:3 * C]
    bq_full = b_attn[C:2 * C]
    in_maps = []
    for core in range(N_CORES):
        b = core // (N_CORES // 2)
        h0 = HPC * (core % (N_CORES // 2))
        ccols = slice(h0 * D, (h0 + 2) * D)
        c2 = slice((h0 + 2) * D, (h0 + 3) * D)
        wq = np.concatenate(
            [Wq[:, ccols], Wk[:, ccols], Wq[:, c2], Wk[:, c2],
             Wv[:, h0 * D:(h0 + 3) * D]], axis=1).astype(ml_dtypes.bfloat16)
        bq = np.zeros((128, 2), np.float32)
        bq[:, 0] = bq_full[ccols]
        bq[0:64, 1] = bq_full[c2]
        wp01 = np.ascontiguousarray(
            W_proj[h0 * D:(h0 + 2) * D, :]).astype(ml_dtypes.bfloat16)
        wp2 = np.ascontiguousarray(
            W_proj[(h0 + 2) * D:(h0 + 3) * D, :]).astype(ml_dtypes.bfloat16)
        in_maps.append({"xt": xts[b], "wq": np.ascontiguousarray(wq),
                        "bq": bq, "wp01": wp01, "wp2": wp2})
    return in_maps


def kernel(x, W_attn, b_attn, W_proj, b_proj):
    x = np.asarray(x, dtype=np.float32)
    W_attn = np.asarray(W_attn, dtype=np.float32)
    b_attn = np.asarray(b_attn, dtype=np.float32)
    W_proj = np.asarray(W_proj, dtype=np.float32)
    b_proj = np.asarray(b_proj, dtype=np.float32)
    B, T, _ = x.shape

    nc = _get_nc(T)
    in_maps = make_core_inputs(x, W_attn, b_attn, W_proj)
    res = None
    for attempt in range(3):
        try:
            res = run_bass_kernel_spmd(nc, in_maps, list(range(N_CORES)))
            break
        except Exception:
            if attempt == 2:
                raise
    global LAST_RUN
    LAST_RUN = res

    gpb = N_CORES // B
    # v-bias folded: softmax weights sum to 1 per row
    b_eff = b_proj + b_attn[2 * C:3 * C] @ W_proj
    out = np.empty((B, T, C), np.float32)
    for b in range(B):
        acc = res.results[b * gpb]["y"].astype(np.float32)
        for g in range(1, gpb):
            acc = acc + res.results[b * gpb + g]["y"]
        out[b] = acc + b_eff[None, :]
    return out
